# revision 7
# baseline (speedup 1.0000x reference)
"""Trainium2 Bass kernel for nn_MoEExpertPool (MoE product-of-experts).

Math (per reference):
  3 modality groups (fs, cb, sp) x 4 experts each = 12 experts.
  Per expert e: h = relu(x @ W1_e + b1_e); mu_e = h @ Wmu_e + bmu_e;
                lv_e = h @ Wlv_e + blv_e.
  Gate per group: w = softmax(x @ Wg + bg) (cb additionally scaled by
  (1 - mean(modality_mask))).  PoE fuse over the 12 experts:
    prec_e = 1 / (exp(lv_e) + eps)
    S2 = sum_e prec_e ; S1 = sum_e w_e * mu_e * prec_e
    mu_fused = S1 / S2 ; logvar_fused = log(1/S2 + eps)

Sharding: pure batch-parallel over 8 cores (512 rows each); every core runs
all 12 experts so S1/S2 are complete per-core — no cross-core reduction.
Device works in a transposed layout (contraction dim on partitions), so
matmuls chain with no on-chip transposes and the per-column biases become
per-partition activation biases.  Matmul operands are float32r: fp32 data
through the PE at 1 cycle/row (4x faster than plain fp32; ~2.3e-4 rel err).
Gates + final divide/log are computed on host (0.01% of FLOPs).
"""

import os
import sys

sys.path.insert(0, "/opt/trn_rl_repo")

import numpy as np

B, D, E, NG = 4096, 2048, 4, 3
NEXP = NG * E            # 12 experts
N_CORES = 8
BC = B // N_CORES        # 512 batch rows per core
MT = D // 128            # 16 output tiles per matmul
KT = D // 128            # 16 contraction tiles
EPS = 1e-8

USE_BF16 = os.environ.get("KERNEL_BF16", "0") == "1"
WSTRIP_BUFS = 8 if USE_BF16 else 6
H_BUFS = 1

_cache = {}


def _bcol(e, j, mt):
    # column in the packed bias tile for expert e, matrix j (0=b1,1=bmu,2=blv)
    return (e * 3 + j) * MT + mt


def _build_nc(reps=1):
    import concourse.mybir as mybir
    import concourse.tile as tile
    from concourse import bacc

    f32 = mybir.dt.float32
    # Matmul operand dtype.  float32r must be declared end-to-end (DRAM and
    # SBUF): the BIR verifier rejects bitcasts into fp32r matmuls, and only
    # gpsimd DMA may cast.  float32r is byte-identical to f32 on the numpy
    # side; the PE rounds internally (~2.3e-4 rel err at K=2048).
    mmdt = mybir.dt.bfloat16 if USE_BF16 else mybir.dt.float32r
    dramdt = mmdt
    AF = mybir.ActivationFunctionType

    nc = bacc.Bacc("TRN2", target_bir_lowering=False)
    xT = nc.dram_tensor("xT", [D, BC], dramdt, kind="ExternalInput")
    # W is pre-tiled on host to [matrix, mt, p, kt, m]: the [128, KT, 128]
    # strip a matmul group needs is a single contiguous 1MB block (8KB per
    # partition), so weight DMA runs at full HBM efficiency instead of
    # 512B-granular gathers.
    W = nc.dram_tensor("W", [NEXP * 3, MT, 128, KT, 128], dramdt, kind="ExternalInput")
    WG = nc.dram_tensor("WG", [NEXP, BC], f32, kind="ExternalInput")
    BIAS = nc.dram_tensor("BIAS", [128, NEXP * 3 * MT], f32, kind="ExternalInput")
    S1 = nc.dram_tensor("S1", [D, BC], f32, kind="ExternalOutput")
    S2 = nc.dram_tensor("S2", [D, BC], f32, kind="ExternalOutput")

    with tile.TileContext(nc) as tc:
        with (
            tc.tile_pool(name="xp", bufs=1) as xp,
            tc.tile_pool(name="hp", bufs=H_BUFS) as hp,
            tc.tile_pool(name="accp", bufs=1) as accp,
            tc.tile_pool(name="wp", bufs=WSTRIP_BUFS) as wp,
            tc.tile_pool(name="gp", bufs=2) as gp,
            tc.tile_pool(name="cp", bufs=1) as cp,
            tc.tile_pool(name="ew", bufs=3) as ew,
            tc.tile_pool(name="psh", bufs=2, space="PSUM") as psh,
            tc.tile_pool(name="psmu", bufs=3, space="PSUM") as psmu,
            tc.tile_pool(name="pslv", bufs=3, space="PSUM") as pslv,
        ):
            xsb = xp.tile([128, KT, BC], mmdt)
            nc.sync.dma_start(xsb[:], xT[:, :].rearrange("(kt p) b -> p kt b", p=128))
            bias_sb = cp.tile([128, NEXP * 3 * MT], f32)
            nc.sync.dma_start(bias_sb[:], BIAS[:, :])
            S1sb = accp.tile([128, MT, BC], f32)
            S2sb = accp.tile([128, MT, BC], f32)

            # reps>1 builds a timing variant that repeats the whole
            # computation in a hardware loop; the outputs are identical
            # every iteration.
            import contextlib

            rep_loop = (
                tc.For_i(0, reps, 1) if reps > 1 else contextlib.nullcontext()
            )
            with rep_loop:
              nc.vector.memset(S1sb[:], 0.0)
              nc.vector.memset(S2sb[:], 0.0)
              for e in range(NEXP):
                  wg_t = gp.tile([128, BC], f32, tag="wg")
                  nc.sync.dma_start(wg_t[:], WG[e : e + 1, :].partition_broadcast(128))

                  h = hp.tile([128, KT, BC], mmdt, tag="h")
                  # layer 1: hT = relu(W1.T @ xT + b1)
                  for mt in range(MT):
                      wst = wp.tile([128, KT, 128], mmdt, tag="wstrip")
                      nc.sync.dma_start(wst[:], W[3 * e, mt])
                      ps = psh.tile([128, BC], f32, tag="psh")
                      for kt in range(KT):
                          nc.tensor.matmul(
                              ps[:],
                              wst[:, kt, :],
                              xsb[:, kt, :],
                              start=(kt == 0),
                              stop=(kt == KT - 1),
                          )
                      nc.scalar.activation(
                          h[:, mt, :], ps[:], AF.Relu,
                          bias=bias_sb[:, _bcol(e, 0, mt) : _bcol(e, 0, mt) + 1],
                      )
                  # layer 2: muT, lvT; fold into PoE partial sums
                  for mt in range(MT):
                      wmu = wp.tile([128, KT, 128], mmdt, tag="wstrip")
                      nc.sync.dma_start(wmu[:], W[3 * e + 1, mt])
                      wlv = wp.tile([128, KT, 128], mmdt, tag="wstrip")
                      nc.sync.dma_start(wlv[:], W[3 * e + 2, mt])
                      pmu = psmu.tile([128, BC], f32, tag="pmu")
                      plv = pslv.tile([128, BC], f32, tag="plv")
                      for kt in range(KT):
                          nc.tensor.matmul(
                              pmu[:], wmu[:, kt, :], h[:, kt, :],
                              start=(kt == 0), stop=(kt == KT - 1),
                          )
                      for kt in range(KT):
                          nc.tensor.matmul(
                              plv[:], wlv[:, kt, :], h[:, kt, :],
                              start=(kt == 0), stop=(kt == KT - 1),
                          )
                      # prec = 1/(exp(lv)+eps) ~= exp(-lv): one scalar op
                      # (blv stored negated in the packed bias).  eps shifts
                      # the result by ~1e-8 relative — far below the gate.
                      prec = ew.tile([128, BC], f32, tag="prec")
                      nc.scalar.activation(
                          prec[:], plv[:], AF.Exp, scale=-1.0,
                          bias=bias_sb[:, _bcol(e, 2, mt) : _bcol(e, 2, mt) + 1],
                      )
                      nc.vector.tensor_add(S2sb[:, mt, :], S2sb[:, mt, :], prec[:])
                      precw = ew.tile([128, BC], f32, tag="precw")
                      nc.vector.tensor_mul(precw[:], prec[:], wg_t[:])
                      mu = ew.tile([128, BC], f32, tag="mu")
                      nc.vector.scalar_tensor_tensor(
                          mu[:], pmu[:],
                          bias_sb[:, _bcol(e, 1, mt) : _bcol(e, 1, mt) + 1],
                          precw[:],
                          op0=mybir.AluOpType.add, op1=mybir.AluOpType.mult,
                      )
                      nc.vector.tensor_add(S1sb[:, mt, :], S1sb[:, mt, :], mu[:])

            for mt in range(MT):
                nc.sync.dma_start(S1[mt * 128 : (mt + 1) * 128, :], S1sb[:, mt, :])
                nc.sync.dma_start(S2[mt * 128 : (mt + 1) * 128, :], S2sb[:, mt, :])

    nc.compile()
    return nc


def _get_nc(reps=1):
    key = ("nc", reps)
    if key not in _cache:
        _cache[key] = _build_nc(reps)
    return _cache[key]


def _host_prep(inputs):
    x = np.asarray(inputs["x"], np.float32)
    mask = np.asarray(inputs["modality_mask"])
    xd = x.astype(np.float64)
    mask_mean = mask.astype(np.float64).mean(axis=1, keepdims=True)  # [B,1]

    if USE_BF16:
        import ml_dtypes
        mmdt_np = ml_dtypes.bfloat16
    else:
        mmdt_np = np.float32

    prefs = ["fs", "cb", "sp"]
    # gate weights [NEXP, B]
    wgate = np.empty((NEXP, B), np.float32)
    for g, pref in enumerate(prefs):
        logits = xd @ np.asarray(inputs[f"{pref}_Wg"], np.float64) + np.asarray(
            inputs[f"{pref}_bg"], np.float64
        )
        logits -= logits.max(axis=1, keepdims=True)
        ex = np.exp(logits)
        w = ex / ex.sum(axis=1, keepdims=True)  # [B, E]
        if pref == "cb":
            w = w * (1.0 - mask_mean)
        wgate[g * E : (g + 1) * E, :] = w.T.astype(np.float32)

    # Weights pre-tiled to [matrix, mt, p, kt, m] so each [128, KT, 128]
    # matmul strip is contiguous in DRAM (see _build_nc).
    Wstack = np.empty((NEXP * 3, MT, 128, KT, 128), mmdt_np)
    bias_arr = np.zeros((128, NEXP * 3 * MT), np.float32)
    for g, pref in enumerate(prefs):
        for e in range(E):
            ge = g * E + e
            for j, nm in enumerate(["W1", "Wmu", "Wlv"]):
                w = np.asarray(inputs[f"{pref}_{nm}"][e])  # [D(in), D(out)]
                Wstack[ge * 3 + j] = (
                    w.reshape(KT, 128, MT, 128).transpose(2, 1, 0, 3).astype(mmdt_np)
                )
            for j, nm in enumerate(["b1", "bmu", "blv"]):
                vec = np.asarray(inputs[f"{pref}_{nm}"][e], np.float32)  # [D]
                if nm == "blv":
                    # the kernel computes prec = exp(-(lv)) via
                    # activation(plv, scale=-1, bias=-blv)
                    vec = -vec
                bias_arr[:, (ge * 3 + j) * MT : (ge * 3 + j + 1) * MT] = vec.reshape(
                    MT, 128
                ).T

    xt = np.ascontiguousarray(x.T.astype(mmdt_np))  # [D, B]
    in_maps = []
    for c in range(N_CORES):
        in_maps.append(
            {
                "xT": np.ascontiguousarray(xt[:, c * BC : (c + 1) * BC]),
                "W": Wstack,
                "WG": np.ascontiguousarray(wgate[:, c * BC : (c + 1) * BC]),
                "BIAS": bias_arr,
            }
        )
    return in_maps


def _finalize(results):
    S1 = np.concatenate([r["S1"] for r in results], axis=1)  # [D, B]
    S2 = np.concatenate([r["S2"] for r in results], axis=1)  # [D, B]
    S2d = S2.astype(np.float64)
    mu_fused = (S1.astype(np.float64) / S2d).T.astype(np.float32)
    logvar_fused = np.log(1.0 / S2d + EPS).T.astype(np.float32)
    return mu_fused, logvar_fused


def kernel(run_kwargs=None, **inputs):
    from concourse.bass_utils import run_bass_kernel_spmd

    nc = _get_nc()
    in_maps = _host_prep(inputs)
    res = run_bass_kernel_spmd(
        nc, in_maps, core_ids=list(range(N_CORES)), **(run_kwargs or {})
    )
    _cache["last_result"] = res
    return _finalize(res.results)



# revision 8
# speedup vs baseline: 1.3234x; 1.3234x over previous
"""Trainium2 Bass kernel for nn_MoEExpertPool — expert x batch hybrid sharding.

Grid: 8 cores = 4 expert-groups x 2 batch-halves.  Core c handles experts
{3g, 3g+1, 3g+2} (g = c % 4) on batch half h = c // 4 (2048 rows).

Per (expert, mt) the weight strip is loaded once and streamed over 4 moving
tiles of 512 batch columns (weight-stationary), so per-core weight DMA is
75.5 MB/rep (vs 604 MB for pure batch-parallel) and LDWEIGHTS cost is
amortized 4x (layer1) / 2x (layer2).

The PoE combine runs on host: each core emits per-expert mu-contribution
(w_e * (mu_e) * prec_e ... stored as (pmu+bmu)*prec*wg) and prec_e = exp(-lv_e)
tiles; host sums the 12 experts per batch half in f64 and finalizes.
All matmul operands are bf16 (rel err ~5e-3, gate is 2e-2).
"""

import contextlib
import os
import sys

sys.path.insert(0, "/opt/trn_rl_repo")

import numpy as np

B, D, E, NG = 4096, 2048, 4, 3
NEXP = NG * E            # 12 experts
N_CORES = 8
BH = B // 2              # 2048 batch rows per core (one half)
EPC = 3                  # experts per core
MT = D // 128            # 16 output tiles
KT = D // 128            # 16 contraction tiles
BT = BH // 512           # 4 moving tiles of 512
EPS = 1e-8

W_BUFS = int(os.environ.get("V3_WBUFS", "6"))

_cache = {}


def _bcol(e, j, mt):
    return (e * 3 + j) * MT + mt


def _build_nc(reps=1):
    import concourse.mybir as mybir
    import concourse.tile as tile
    from concourse import bacc

    f32 = mybir.dt.float32
    mmdt = mybir.dt.bfloat16
    AF = mybir.ActivationFunctionType

    nc = bacc.Bacc("TRN2", target_bir_lowering=False)
    xT = nc.dram_tensor("xT", [D, BH], mmdt, kind="ExternalInput")
    # pre-tiled weight strips: [matrix, mt, p, kt, m], each strip contiguous
    W = nc.dram_tensor("W", [EPC * 3, MT, 128, KT, 128], mmdt, kind="ExternalInput")
    WG = nc.dram_tensor("WG", [EPC, BH], f32, kind="ExternalInput")
    BIAS = nc.dram_tensor("BIAS", [128, EPC * 3 * MT], f32, kind="ExternalInput")
    MUC = nc.dram_tensor("MUC", [EPC, D, BH], f32, kind="ExternalOutput")
    PRC = nc.dram_tensor("PRC", [EPC, D, BH], f32, kind="ExternalOutput")

    with tile.TileContext(nc) as tc:
        with (
            tc.tile_pool(name="xp", bufs=1) as xp,
            tc.tile_pool(name="hp", bufs=1) as hp,
            tc.tile_pool(name="wp", bufs=W_BUFS) as wp,
            tc.tile_pool(name="gp", bufs=2) as gp,
            tc.tile_pool(name="cp", bufs=1) as cp,
            tc.tile_pool(name="ew", bufs=4) as ew,
            tc.tile_pool(name="psh", bufs=1, space="PSUM") as psh,
            tc.tile_pool(name="psmu", bufs=1, space="PSUM") as psmu,
            tc.tile_pool(name="pslv", bufs=1, space="PSUM") as pslv,
        ):
            xsb = xp.tile([128, KT, BH], mmdt)
            nc.sync.dma_start(xsb[:], xT[:, :].rearrange("(kt p) b -> p kt b", p=128))
            bias_sb = cp.tile([128, EPC * 3 * MT], f32)
            nc.sync.dma_start(bias_sb[:], BIAS[:, :])

            rep_loop = (
                tc.For_i(0, reps, 1) if reps > 1 else contextlib.nullcontext()
            )
            with rep_loop:
                for e in range(EPC):
                    wg_t = gp.tile([128, BH], f32, tag="wg")
                    nc.sync.dma_start(
                        wg_t[:], WG[e : e + 1, :].partition_broadcast(128)
                    )

                    h = hp.tile([128, KT, BH], mmdt, tag="h")
                    # layer 1: hT = relu(W1.T @ xT + b1), weight-stationary
                    # over 4 moving tiles per (mt, kt)
                    for mt in range(MT):
                        wst = wp.tile([128, KT, 128], mmdt, tag="wstrip")
                        nc.sync.dma_start(wst[:], W[3 * e, mt])
                        ps4 = psh.tile([128, BT, 512], f32, tag="psh")
                        for kt in range(KT):
                            for bt in range(BT):
                                nc.tensor.matmul(
                                    ps4[:, bt, :],
                                    wst[:, kt, :],
                                    xsb[:, kt, bt * 512 : (bt + 1) * 512],
                                    start=(kt == 0),
                                    stop=(kt == KT - 1),
                                )
                        for bt in range(BT):
                            nc.scalar.activation(
                                h[:, mt, bt * 512 : (bt + 1) * 512],
                                ps4[:, bt, :],
                                AF.Relu,
                                bias=bias_sb[:, _bcol(e, 0, mt) : _bcol(e, 0, mt) + 1],
                            )
                    # layer 2: mu/lv in bt-pairs (weights reused x2 per load,
                    # 4 psum banks: pmu2 + plv2)
                    for mt in range(MT):
                        wmu = wp.tile([128, KT, 128], mmdt, tag="wstrip")
                        nc.sync.dma_start(wmu[:], W[3 * e + 1, mt])
                        wlv = wp.tile([128, KT, 128], mmdt, tag="wstrip")
                        nc.sync.dma_start(wlv[:], W[3 * e + 2, mt])
                        for btp in range(0, BT, 2):
                            pmu2 = psmu.tile([128, 2, 512], f32, tag="pmu")
                            plv2 = pslv.tile([128, 2, 512], f32, tag="plv")
                            for kt in range(KT):
                                for j in range(2):
                                    nc.tensor.matmul(
                                        pmu2[:, j, :],
                                        wmu[:, kt, :],
                                        h[:, kt, (btp + j) * 512 : (btp + j + 1) * 512],
                                        start=(kt == 0),
                                        stop=(kt == KT - 1),
                                    )
                            for kt in range(KT):
                                for j in range(2):
                                    nc.tensor.matmul(
                                        plv2[:, j, :],
                                        wlv[:, kt, :],
                                        h[:, kt, (btp + j) * 512 : (btp + j + 1) * 512],
                                        start=(kt == 0),
                                        stop=(kt == KT - 1),
                                    )
                            for j in range(2):
                                bt = btp + j
                                # prec = exp(-(plv + blv)); blv pre-negated
                                prec = ew.tile([128, 512], f32, tag="prec")
                                nc.scalar.activation(
                                    prec[:], plv2[:, j, :], AF.Exp, scale=-1.0,
                                    bias=bias_sb[
                                        :, _bcol(e, 2, mt) : _bcol(e, 2, mt) + 1
                                    ],
                                )
                                nc.sync.dma_start(
                                    PRC[
                                        e,
                                        mt * 128 : (mt + 1) * 128,
                                        bt * 512 : (bt + 1) * 512,
                                    ],
                                    prec[:],
                                )
                                precw = ew.tile([128, 512], f32, tag="precw")
                                nc.vector.tensor_mul(
                                    precw[:], prec[:],
                                    wg_t[:, bt * 512 : (bt + 1) * 512],
                                )
                                mu = ew.tile([128, 512], f32, tag="mu")
                                nc.vector.scalar_tensor_tensor(
                                    mu[:], pmu2[:, j, :],
                                    bias_sb[:, _bcol(e, 1, mt) : _bcol(e, 1, mt) + 1],
                                    precw[:],
                                    op0=mybir.AluOpType.add,
                                    op1=mybir.AluOpType.mult,
                                )
                                nc.sync.dma_start(
                                    MUC[
                                        e,
                                        mt * 128 : (mt + 1) * 128,
                                        bt * 512 : (bt + 1) * 512,
                                    ],
                                    mu[:],
                                )

    nc.compile()
    return nc


def _get_nc(reps=1):
    key = ("nc", reps)
    if key not in _cache:
        _cache[key] = _build_nc(reps)
    return _cache[key]


def _host_prep(inputs):
    import ml_dtypes

    bf16 = ml_dtypes.bfloat16
    x = np.asarray(inputs["x"], np.float32)
    mask = np.asarray(inputs["modality_mask"])
    xd = x.astype(np.float64)
    mask_mean = mask.astype(np.float64).mean(axis=1, keepdims=True)

    prefs = ["fs", "cb", "sp"]
    wgate = np.empty((NEXP, B), np.float32)
    for g, pref in enumerate(prefs):
        logits = xd @ np.asarray(inputs[f"{pref}_Wg"], np.float64) + np.asarray(
            inputs[f"{pref}_bg"], np.float64
        )
        logits -= logits.max(axis=1, keepdims=True)
        ex = np.exp(logits)
        w = ex / ex.sum(axis=1, keepdims=True)
        if pref == "cb":
            w = w * (1.0 - mask_mean)
        wgate[g * E : (g + 1) * E, :] = w.T.astype(np.float32)

    # strips pre-tiled [matrix, mt, p, kt, m]; biases packed per group of 3
    # experts (blv negated for the exp(-lv) activation)
    Wall = np.empty((NEXP, 3, MT, 128, KT, 128), bf16)
    ball = np.zeros((NEXP, 3, 128, MT), np.float32)
    for g, pref in enumerate(prefs):
        for e in range(E):
            ge = g * E + e
            for j, nm in enumerate(["W1", "Wmu", "Wlv"]):
                w = np.asarray(inputs[f"{pref}_{nm}"][e])
                Wall[ge, j] = (
                    w.reshape(KT, 128, MT, 128).transpose(2, 1, 0, 3).astype(bf16)
                )
            for j, nm in enumerate(["b1", "bmu", "blv"]):
                vec = np.asarray(inputs[f"{pref}_{nm}"][e], np.float32)
                if nm == "blv":
                    vec = -vec
                ball[ge, j] = vec.reshape(MT, 128).T

    xt = np.ascontiguousarray(x.T.astype(bf16))  # [D, B]
    in_maps = []
    for c in range(N_CORES):
        g, half = c % 4, c // 4
        exps = [3 * g, 3 * g + 1, 3 * g + 2]
        bias_arr = np.zeros((128, EPC * 3 * MT), np.float32)
        for ei, ge in enumerate(exps):
            for j in range(3):
                bias_arr[:, (ei * 3 + j) * MT : (ei * 3 + j + 1) * MT] = ball[ge, j]
        in_maps.append(
            {
                "xT": np.ascontiguousarray(xt[:, half * BH : (half + 1) * BH]),
                "W": np.ascontiguousarray(
                    Wall[exps].reshape(EPC * 3, MT, 128, KT, 128)
                ),
                "WG": np.ascontiguousarray(
                    wgate[exps, half * BH : (half + 1) * BH]
                ),
                "BIAS": bias_arr,
            }
        )
    return in_maps


def _finalize(results):
    mu_fused = np.empty((B, D), np.float32)
    lv_fused = np.empty((B, D), np.float32)
    for half in range(2):
        S1 = np.zeros((D, BH), np.float64)
        S2 = np.zeros((D, BH), np.float64)
        for g in range(4):
            r = results[half * 4 + g]
            S1 += r["MUC"].astype(np.float64).sum(axis=0)
            S2 += r["PRC"].astype(np.float64).sum(axis=0)
        sl = slice(half * BH, (half + 1) * BH)
        mu_fused[sl] = (S1 / S2).T.astype(np.float32)
        lv_fused[sl] = np.log(1.0 / S2 + EPS).T.astype(np.float32)
    return mu_fused, lv_fused


def kernel(run_kwargs=None, **inputs):
    from concourse.bass_utils import run_bass_kernel_spmd

    nc = _get_nc()
    in_maps = _host_prep(inputs)
    res = run_bass_kernel_spmd(
        nc, in_maps, core_ids=list(range(N_CORES)), **(run_kwargs or {})
    )
    _cache["last_result"] = res
    return _finalize(res.results)


# revision 10
# speedup vs baseline: 1.3487x; 1.0191x over previous
"""Trainium2 Bass kernel for nn_MoEExpertPool — expert x batch hybrid sharding.

Grid: 8 cores = 4 expert-groups x 2 batch-halves.  Core c handles experts
{3g, 3g+1, 3g+2} (g = c % 4) on batch half h = c // 4 (2048 rows).

Per (expert, mt) the weight strip is loaded once and streamed over 4 moving
tiles of 512 batch columns (weight-stationary 4x in both layers), so per-core
weight DMA is 75.5 MB/rep (vs 604 MB for pure batch-parallel) and LDWEIGHTS
cost is amortized.

The PoE combine runs on host: each core emits per-expert mu-contribution
(w_e * (mu_e) * prec_e ... stored as (pmu+bmu)*prec*wg) and prec_e = exp(-lv_e)
tiles; host sums the 12 experts per batch half in f64 and finalizes.
All matmul operands are bf16 (rel err ~5e-3, gate is 2e-2).
"""

import contextlib
import os
import sys

sys.path.insert(0, "/opt/trn_rl_repo")

import numpy as np

B, D, E, NG = 4096, 2048, 4, 3
NEXP = NG * E            # 12 experts
N_CORES = 8
BH = B // 2              # 2048 batch rows per core (one half)
EPC = 3                  # experts per core
MT = D // 128            # 16 output tiles
KT = D // 128            # 16 contraction tiles
BT = BH // 512           # 4 moving tiles of 512
EPS = 1e-8

W_BUFS = int(os.environ.get("V3_WBUFS", "6"))

_cache = {}


def _bcol(e, j, mt):
    return (e * 3 + j) * MT + mt


def _build_nc(reps=1):
    import concourse.mybir as mybir
    import concourse.tile as tile
    from concourse import bacc

    f32 = mybir.dt.float32
    mmdt = mybir.dt.bfloat16
    AF = mybir.ActivationFunctionType

    nc = bacc.Bacc("TRN2", target_bir_lowering=False)
    xT = nc.dram_tensor("xT", [D, BH], mmdt, kind="ExternalInput")
    # pre-tiled weight strips: [matrix, mt, p, kt, m], each strip contiguous
    W = nc.dram_tensor("W", [EPC * 3, MT, 128, KT, 128], mmdt, kind="ExternalInput")
    WG = nc.dram_tensor("WG", [EPC, BH], f32, kind="ExternalInput")
    BIAS = nc.dram_tensor("BIAS", [128, EPC * 3 * MT], f32, kind="ExternalInput")
    MUC = nc.dram_tensor("MUC", [EPC, D, BH], f32, kind="ExternalOutput")
    PRC = nc.dram_tensor("PRC", [EPC, D, BH], f32, kind="ExternalOutput")

    with tile.TileContext(nc) as tc:
        with (
            tc.tile_pool(name="xp", bufs=1) as xp,
            tc.tile_pool(name="hp", bufs=1) as hp,
            tc.tile_pool(name="wp", bufs=W_BUFS) as wp,
            tc.tile_pool(name="gp", bufs=2) as gp,
            tc.tile_pool(name="cp", bufs=1) as cp,
            tc.tile_pool(name="ew", bufs=4) as ew,
            # one 8-bank PSUM pool: [128, 4, 512] quad tiles, double-buffered.
            # Every matmul chain streams 4 moving tiles against one weight
            # tile (weight-stationary x4 in both layers).
            tc.tile_pool(name="psq", bufs=2, space="PSUM") as psq,
        ):
            xsb = xp.tile([128, KT, BH], mmdt)
            nc.sync.dma_start(xsb[:], xT[:, :].rearrange("(kt p) b -> p kt b", p=128))
            bias_sb = cp.tile([128, EPC * 3 * MT], f32)
            nc.sync.dma_start(bias_sb[:], BIAS[:, :])

            rep_loop = (
                tc.For_i(0, reps, 1) if reps > 1 else contextlib.nullcontext()
            )
            with rep_loop:
                for e in range(EPC):
                    wg_t = gp.tile([128, BH], f32, tag="wg")
                    nc.sync.dma_start(
                        wg_t[:], WG[e : e + 1, :].partition_broadcast(128)
                    )

                    h = hp.tile([128, KT, BH], mmdt, tag="h")
                    # layer 1: hT = relu(W1.T @ xT + b1), weight-stationary
                    # over 4 moving tiles per (mt, kt)
                    for mt in range(MT):
                        wst = wp.tile([128, KT, 128], mmdt, tag="wstrip")
                        nc.sync.dma_start(wst[:], W[3 * e, mt])
                        ps4 = psq.tile([128, BT, 512], f32, tag="quad")
                        for kt in range(KT):
                            for bt in range(BT):
                                nc.tensor.matmul(
                                    ps4[:, bt, :],
                                    wst[:, kt, :],
                                    xsb[:, kt, bt * 512 : (bt + 1) * 512],
                                    start=(kt == 0),
                                    stop=(kt == KT - 1),
                                )
                        for bt in range(BT):
                            nc.scalar.activation(
                                h[:, mt, bt * 512 : (bt + 1) * 512],
                                ps4[:, bt, :],
                                AF.Relu,
                                bias=bias_sb[:, _bcol(e, 0, mt) : _bcol(e, 0, mt) + 1],
                            )
                    # layer 2: mu/lv in bt-pairs (weights reused x2 per load,
                    # 4 psum banks: pmu2 + plv2)
                    for mt in range(MT):
                        wmu = wp.tile([128, KT, 128], mmdt, tag="wstrip")
                        nc.sync.dma_start(wmu[:], W[3 * e + 1, mt])
                        wlv = wp.tile([128, KT, 128], mmdt, tag="wstrip")
                        nc.sync.dma_start(wlv[:], W[3 * e + 2, mt])
                        # lv chain FIRST: its consumers (exp, precw) overlap
                        # with the mu chain; stt fires right after the mu
                        # chain stops -> the next quad allocation never waits.
                        plv4 = psq.tile([128, BT, 512], f32, tag="quad")
                        for kt in range(KT):
                            for bt in range(BT):
                                nc.tensor.matmul(
                                    plv4[:, bt, :],
                                    wlv[:, kt, :],
                                    h[:, kt, bt * 512 : (bt + 1) * 512],
                                    start=(kt == 0),
                                    stop=(kt == KT - 1),
                                )
                        precws = []
                        for bt in range(BT):
                            # prec = exp(-(plv + blv)); blv pre-negated
                            prec = ew.tile([128, 512], f32, tag="prec")
                            nc.scalar.activation(
                                prec[:], plv4[:, bt, :], AF.Exp, scale=-1.0,
                                bias=bias_sb[
                                    :, _bcol(e, 2, mt) : _bcol(e, 2, mt) + 1
                                ],
                            )
                            nc.sync.dma_start(
                                PRC[
                                    e,
                                    mt * 128 : (mt + 1) * 128,
                                    bt * 512 : (bt + 1) * 512,
                                ],
                                prec[:],
                            )
                            precw = ew.tile([128, 512], f32, tag="precw")
                            nc.vector.tensor_mul(
                                precw[:], prec[:],
                                wg_t[:, bt * 512 : (bt + 1) * 512],
                            )
                            precws.append(precw)
                        pmu4 = psq.tile([128, BT, 512], f32, tag="quad")
                        for kt in range(KT):
                            for bt in range(BT):
                                nc.tensor.matmul(
                                    pmu4[:, bt, :],
                                    wmu[:, kt, :],
                                    h[:, kt, bt * 512 : (bt + 1) * 512],
                                    start=(kt == 0),
                                    stop=(kt == KT - 1),
                                )
                        for bt in range(BT):
                            mu = ew.tile([128, 512], f32, tag="mu")
                            nc.vector.scalar_tensor_tensor(
                                mu[:], pmu4[:, bt, :],
                                bias_sb[:, _bcol(e, 1, mt) : _bcol(e, 1, mt) + 1],
                                precws[bt][:],
                                op0=mybir.AluOpType.add,
                                op1=mybir.AluOpType.mult,
                            )
                            nc.sync.dma_start(
                                MUC[
                                    e,
                                    mt * 128 : (mt + 1) * 128,
                                    bt * 512 : (bt + 1) * 512,
                                ],
                                mu[:],
                            )

    nc.compile()
    return nc


def _get_nc(reps=1):
    key = ("nc", reps)
    if key not in _cache:
        _cache[key] = _build_nc(reps)
    return _cache[key]


def _host_prep(inputs):
    import ml_dtypes

    bf16 = ml_dtypes.bfloat16
    x = np.asarray(inputs["x"], np.float32)
    mask = np.asarray(inputs["modality_mask"])
    xd = x.astype(np.float64)
    mask_mean = mask.astype(np.float64).mean(axis=1, keepdims=True)

    prefs = ["fs", "cb", "sp"]
    wgate = np.empty((NEXP, B), np.float32)
    for g, pref in enumerate(prefs):
        logits = xd @ np.asarray(inputs[f"{pref}_Wg"], np.float64) + np.asarray(
            inputs[f"{pref}_bg"], np.float64
        )
        logits -= logits.max(axis=1, keepdims=True)
        ex = np.exp(logits)
        w = ex / ex.sum(axis=1, keepdims=True)
        if pref == "cb":
            w = w * (1.0 - mask_mean)
        wgate[g * E : (g + 1) * E, :] = w.T.astype(np.float32)

    # strips pre-tiled [matrix, mt, p, kt, m]; biases packed per group of 3
    # experts (blv negated for the exp(-lv) activation)
    Wall = np.empty((NEXP, 3, MT, 128, KT, 128), bf16)
    ball = np.zeros((NEXP, 3, 128, MT), np.float32)
    for g, pref in enumerate(prefs):
        for e in range(E):
            ge = g * E + e
            for j, nm in enumerate(["W1", "Wmu", "Wlv"]):
                w = np.asarray(inputs[f"{pref}_{nm}"][e])
                Wall[ge, j] = (
                    w.reshape(KT, 128, MT, 128).transpose(2, 1, 0, 3).astype(bf16)
                )
            for j, nm in enumerate(["b1", "bmu", "blv"]):
                vec = np.asarray(inputs[f"{pref}_{nm}"][e], np.float32)
                if nm == "blv":
                    vec = -vec
                ball[ge, j] = vec.reshape(MT, 128).T

    xt = np.ascontiguousarray(x.T.astype(bf16))  # [D, B]
    in_maps = []
    for c in range(N_CORES):
        g, half = c % 4, c // 4
        exps = [3 * g, 3 * g + 1, 3 * g + 2]
        bias_arr = np.zeros((128, EPC * 3 * MT), np.float32)
        for ei, ge in enumerate(exps):
            for j in range(3):
                bias_arr[:, (ei * 3 + j) * MT : (ei * 3 + j + 1) * MT] = ball[ge, j]
        in_maps.append(
            {
                "xT": np.ascontiguousarray(xt[:, half * BH : (half + 1) * BH]),
                "W": np.ascontiguousarray(
                    Wall[exps].reshape(EPC * 3, MT, 128, KT, 128)
                ),
                "WG": np.ascontiguousarray(
                    wgate[exps, half * BH : (half + 1) * BH]
                ),
                "BIAS": bias_arr,
            }
        )
    return in_maps


def _finalize(results):
    mu_fused = np.empty((B, D), np.float32)
    lv_fused = np.empty((B, D), np.float32)
    for half in range(2):
        S1 = np.zeros((D, BH), np.float64)
        S2 = np.zeros((D, BH), np.float64)
        for g in range(4):
            r = results[half * 4 + g]
            S1 += r["MUC"].astype(np.float64).sum(axis=0)
            S2 += r["PRC"].astype(np.float64).sum(axis=0)
        sl = slice(half * BH, (half + 1) * BH)
        mu_fused[sl] = (S1 / S2).T.astype(np.float32)
        lv_fused[sl] = np.log(1.0 / S2 + EPS).T.astype(np.float32)
    return mu_fused, lv_fused


def kernel(run_kwargs=None, **inputs):
    from concourse.bass_utils import run_bass_kernel_spmd

    nc = _get_nc()
    in_maps = _host_prep(inputs)
    res = run_bass_kernel_spmd(
        nc, in_maps, core_ids=list(range(N_CORES)), **(run_kwargs or {})
    )
    _cache["last_result"] = res
    return _finalize(res.results)
